# revision 31
# baseline (speedup 1.0000x reference)
"""Adaptive-input-embedding Bass kernel for one TRN2 chip (8 NeuronCores).

Strategy: token-parallel across the 8 cores — the 32768 tokens are grouped by
bucket, sorted by table index, and dealt as contiguous runs to the cores, so
every core processes ~4096 tokens with identical compile-time structure.
Tables and projection matrices are replicated, except that each core receives
only a <=32k-row *window* of the 237k-row tail-bucket table covering its run
(this keeps gather indices within int16 for the DMA-gather engine).

Device side: per bucket, dma_gather(transpose=True) calls (<=768 indices
each) pull the bf16 embedding rows into SBUF already transposed ([d, tokens]
chunks, i.e. matmul lhsT layout); per 128-token tile the d/128 chunk matmuls
accumulate into PSUM fp32 against the resident bf16 projection chunks; PSUM
is copied to SBUF (alternating DVE/ACT) and written out with large
contiguous partition-major DMA stores (alternating the two HWDGE rings).
The host scatters the returned rows to their token positions while
assembling the full output (the unshard step).
"""

import sys

import numpy as np

try:
    import concourse  # noqa: F401
except ImportError:
    sys.path.insert(0, "/opt/trn_rl_repo")

import ml_dtypes
from concourse import bacc, mybir, tile
from concourse.bass_utils import run_bass_kernel_spmd

BUCKETS = (0, 300, 3000, 30000, 267734)
SIZES = [BUCKETS[i + 1] - BUCKETS[i] for i in range(4)]
D = 1024
DS = [1024, 512, 256, 128]  # embedding dim per bucket
KS = [8, 4, 2, 1]  # 128-chunks per bucket
WOFF = [0, 8, 12, 14]  # chunk offset of each bucket in wcat
NCHUNK = 15
SUB = 32768  # rows addressable by one int16 gather call
NCORES = 8
SEQ = 4096
NTOK = NCORES * SEQ
P = 128
GB = 8  # tiles per store batch

MODE = "seq_bf16"

_BF16 = ml_dtypes.bfloat16

_cache: dict = {}


def _r16(v):
    return -(-int(v) // 16) * 16


def _r128(v):
    return -(-int(v) // 128) * 128


class Plan:
    pass


def _plan(x):
    """Global bucketing + even dealing of each bucket across the cores.

    Bucket 3 (237k rows) is dealt as contiguous runs of the index-sorted
    token list, so each core's gather indices span < 32k table rows and fit
    int16 against a per-core window of the table (passed as that core's e3
    input). Produces identical compile-time structure for all cores."""
    xf = x.reshape(-1).astype(np.int64)
    assert xf.shape[0] == NTOK
    bkt = np.searchsorted(np.asarray(BUCKETS), xf, side="right") - 1
    bkt = np.clip(bkt, 0, 3)
    loc = xf - np.asarray(BUCKETS)[bkt]

    # per-(bucket, core) token positions: sort by table index, deal
    # contiguous runs (counts differ by <=1, spans stay narrow for bucket 3)
    per_core_pos = {}
    wbase = np.zeros((4, NCORES), np.int64)  # per-core table window base
    alloc = [0] * 4
    wrows = [0] * 4  # table window rows (compile-time shape)
    for b in range(4):
        pos = np.nonzero(bkt == b)[0]
        pos = pos[np.argsort(loc[pos], kind="stable")]
        n = pos.size
        cnt = np.full(NCORES, n // NCORES)
        cnt[: n % NCORES] += 1
        cuts = np.concatenate([[0], np.cumsum(cnt)])

        def spans(cuts_):
            sp, mx = 0, 0
            for c in range(NCORES):
                pc = pos[cuts_[c] : cuts_[c + 1]]
                if pc.size:
                    sp = max(sp, int(loc[pc[-1]] - loc[pc[0]]) + 1)
                    mx = max(mx, pc.size)
            return sp, mx

        span, mxc = spans(cuts)
        if b == 3 and span > SUB:
            # skewed distribution: balanced cuts straddle >32k-row ranges;
            # fall back to fixed 32k-row boundary cuts (unbalanced counts
            # but indices stay int16 against each core's window)
            edges = np.searchsorted(loc[pos], np.arange(1, NCORES) * SUB)
            cuts = np.concatenate([[0], edges, [n]])
            span, mxc = spans(cuts)
        for c in range(NCORES):
            pc = pos[cuts[c] : cuts[c + 1]]
            per_core_pos[(b, c)] = pc
            if pc.size:
                wbase[b, c] = loc[pc[0]]
        alloc[b] = int(_r16(mxc))
        wrows[b] = min(span if b == 3 else SIZES[b], SIZES[b])
        wrows[b] = max(wrows[b], 1)
        assert wrows[b] <= SUB, (b, wrows[b])
        if b < 3:
            wbase[b] = 0

    # slot layout: one 128-aligned block per bucket
    segs = []  # (bucket, o_slot, n_alloc, num_idxs)
    blocks = []
    o = 0
    for b in range(4):
        ni = _r128(alloc[b])
        segs.append((b, o, alloc[b], ni))
        blocks.append((o, ni))
        o += ni
    ntot = o

    p = Plan()
    p.segs, p.blocks, p.ntot = segs, blocks, ntot
    p.t_total = ntot // P
    p.alloc = alloc
    p.wrows = wrows
    p.wbase = wbase

    gidx = np.zeros((NCORES, P, ntot // 16), np.int16)
    rowpos = np.full((NCORES, ntot), -1, np.int64)  # slot -> global token pos
    for b, o, na, ni in segs:
        for c in range(NCORES):
            pos = per_core_pos[(b, c)]
            n = pos.size
            li = np.zeros(na, np.int64)
            li[:n] = loc[pos] - wbase[b, c]
            rowpos[c, o : o + n] = pos
            ii = np.arange(na)
            cols = o // 16 + ii // 16
            rows = ii % 16
            for g in range(8):  # replicate across the 8 groups of 16 partitions
                gidx[c, g * 16 + rows, cols] = li.astype(np.int16)
    p.gidx, p.rowpos = gidx, rowpos
    return p


def _build(plan, mode=MODE, repeat=1, loop_n=None, b3_first=True, split_w=True, gbatch=16, zbufs=3, psbufs=4, store_split=True, tail_dve=True, ab_no_store=False, ab_no_gather=False, ab_no_mm=False, ab_no_copy=False, g_transpose=True, g_cap=768, g_sp=True, g_q=4, g_head=128, ps_half=False, tail_split=0, ab_same_w=False, copy3=False, gsched=(1, 1, 2, 4, 8, 8, 8, 8, 8, 8)):
    """Build + compile the SPMD Bass program.

    repeat>1 re-emits the whole body; loop_n wraps the body in a HW For_i
    loop (both used only for differential timing)."""
    ntot, t_total = plan.ntot, plan.t_total
    bf16 = mybir.dt.bfloat16
    f32 = mybir.dt.float32
    odt = bf16 if mode.endswith("bf16") else f32

    assert 8 % g_q == 0, "g_q must divide the 8 DMASW sem lanes"
    nc = bacc.Bacc(None, target_bir_lowering=False, num_swdge_queues=g_q)
    # Pool-engine DMA instructions take DMASW sem lanes round-robin (8 lanes);
    # the sim (and sane HW sync) requires each lane to stay on ONE SWDGE
    # queue, so derive each gather's queue from the lane it will be assigned.
    pool_dma_n = [0]

    def _pool_q():
        q = (pool_dma_n[0] % 8) % g_q
        pool_dma_n[0] += 1
        return q
    e_d = [
        nc.declare_dram_parameter(f"e{b}", [plan.wrows[b], DS[b]], bf16, isOutput=False)
        for b in range(4)
    ]
    wcat_d = nc.declare_dram_parameter("wcat", [P, NCHUNK * D], bf16, isOutput=False)
    gidx_d = nc.declare_dram_parameter("gidx", [P, ntot // 16], mybir.dt.int16, isOutput=False)
    # partition-major: slot s lives at out[s % 128, s // 128, :] so each
    # partition's store stream is contiguous (few, large descriptors)
    out_d = nc.declare_dram_parameter("out", [P, t_total, D], odt, isOutput=True)

    bbase = [blk[0] for blk in plan.blocks]
    bslots = [blk[1] for blk in plan.blocks]

    with tile.TileContext(nc) as tc:
        with (
            tc.tile_pool(name="const", bufs=1) as cp,
            tc.tile_pool(name="gbuf", bufs=1) as gp,
            tc.tile_pool(name="zbuf", bufs=zbufs) as zp,
            tc.tile_pool(name="ps", bufs=psbufs, space="PSUM") as pp,
        ):
            gidx = cp.tile([P, ntot // 16], mybir.dt.int16)
            nc.gpsimd.dma_start(out=gidx[:], in_=gidx_d[:])
            _pool_q()  # gidx load consumes a DMASW lane
            wcat = cp.tile([P, NCHUNK * D], bf16)
            if split_w:
                # W3 first: bucket-3 tiles are processed first and its W slice
                # is small, so the first matmuls aren't gated on the full load
                upfront = (3,) if split_w == 3 else (3, 2, 1, 0)
                for b in upfront:
                    sl = slice(WOFF[b] * D, (WOFF[b] + KS[b]) * D)
                    eng = nc.sync if (split_w is True or split_w in (1, 3) or b >= 2) else nc.scalar
                    eng.dma_start(out=wcat[:, sl], in_=wcat_d[:, sl])
            else:
                nc.sync.dma_start(out=wcat[:], in_=wcat_d[:])

            gt = [
                gp.tile([P, KS[b], bslots[b]], bf16, name=f"G{b}", tag=f"G{b}")
                if bslots[b]
                else None
                for b in range(4)
            ]


            def bucket_of_tile(t):
                slot = t * P
                for b in range(4):
                    if bbase[b] <= slot < bbase[b] + bslots[b]:
                        return b
                raise AssertionError(t)

            def body(_iv=None, unroll=1):
                if ab_no_gather and not ab_no_mm:
                    for b in range(4):  # dummy-allocate gt for the ablation
                        nc.vector.memset(gt[b][:, :, :1], 0)
                deferred_w = []
                if split_w == 3:
                    for b in (2, 1, 0):
                        sl = slice(WOFF[b] * D, (WOFF[b] + KS[b]) * D)
                        deferred_w.append(sl)
                GCAP = g_cap  # >=1024 idxs in one SWDGE gather wedges the device
                gcall = 0
                gsegs = (
                    sorted(plan.segs, key=lambda e: e[0] != 3)
                    if b3_first
                    else plan.segs
                )

                def chunks_for(b, ni, first):
                    if KS[b] != 1:
                        return [(0, ni)]
                    out, k = [], 0
                    if first and g_head:
                        # one small head call per SWDGE queue so the first
                        # tiles land fast on all queues concurrently
                        for _ in range(g_q):
                            if k + g_head > ni:
                                break
                            out.append((k, g_head))
                            k += g_head
                    while k < ni:
                        nk = min(GCAP, ni - k)
                        out.append((k, nk))
                        k += nk
                    return out

                for si, (b, o, na, ni) in enumerate(gsegs):
                    if ab_no_gather:
                        break
                    for k, nk in chunks_for(b, ni, si == 0):
                        ok = o + k
                        o_local = ok - bbase[b]
                        if g_transpose:
                            out_ap = gt[b][:, :, o_local : o_local + nk]
                        else:
                            gnt = gp.tile(
                                [P, nk // P, DS[b]],
                                bf16,
                                name=f"GN{gcall}",
                                tag=f"GN{gcall}",
                            )
                            out_ap = gnt[:]
                        nc.gpsimd.dma_gather(
                            out_ap=out_ap,
                            in_ap=e_d[b][:],
                            idxs_ap=gidx[:, ok // 16 : ok // 16 + nk // 16],
                            num_idxs=nk,
                            num_idxs_reg=nk,
                            elem_size=DS[b],
                            transpose=g_transpose,
                            single_packet=g_sp,
                            queue_num=_pool_q(),
                        )
                        gcall += 1

                # bucket-3 tiles first (largest block, cheapest W), then 0..2
                t3 = bbase[3] // P
                if b3_first:
                    order = list(range(t3, t_total)) + list(range(0, t3))
                else:
                    order = list(range(t_total))
                oi = 0
                batch_i = 0
                while oi < len(order):
                    t = order[oi]
                    rem = len(order) - oi
                    if gsched is not None:
                        eff_gb = gsched[batch_i] if batch_i < len(gsched) else gsched[-1]
                    else:
                        eff_gb = 2 if (tail_split and rem <= tail_split) else gbatch
                    batch_i += 1
                    gb = 1
                    for g in range(1, min(eff_gb, rem)):
                        if order[oi + g] == t + g:
                            gb += 1
                        else:
                            break
                    zt = zp.tile([P, gb, D], odt, tag="z")
                    if ab_no_mm or ab_no_copy:
                        nc.vector.memset(zt[:, :, :1], 0)
                    for g in range(gb if not ab_no_mm else 0):
                        tt = t + g
                        b = bucket_of_tile(tt)
                        ts0 = tt * P - bbase[b]
                        kb = KS[b]
                        if ps_half:
                            ps0 = pp.tile([P, 512], f32, tag="ps0")
                            ps1 = pp.tile([P, 512], f32, tag="ps1")
                            pss = (ps0, ps1)
                            for c in range(kb):
                                lhsT = gt[b][:, c, ts0 : ts0 + P]
                                for h in range(2):
                                    nc.tensor.matmul(
                                        out=pss[h][:, :],
                                        lhsT=lhsT,
                                        rhs=wcat[:, (WOFF[b] + c) * D + h * 512 :][:, :512],
                                        start=(c == 0),
                                        stop=(c == kb - 1),
                                    )
                            if not ab_no_copy:
                                nc.vector.tensor_copy(out=zt[:, g, :512], in_=ps0[:])
                                nc.scalar.copy(out=zt[:, g, 512:], in_=ps1[:])
                            oi_done = True
                        else:
                            ps = pp.tile([P, D], f32, tag="ps")
                            for c in range(kb):
                                lhsT = (
                                    gt[3][:, 0, 0:P]
                                    if ab_same_w
                                    else gt[b][:, c, ts0 : ts0 + P]
                                )
                                for h in range(2):
                                    nc.tensor.matmul(
                                        out=ps[:, h * 512 : (h + 1) * 512],
                                        lhsT=lhsT,
                                        rhs=wcat[:, (WOFF[b] + c) * D + h * 512 :][:, :512],
                                        start=(c == 0),
                                        stop=(c == kb - 1),
                                    )
                            last_batch = oi + gb >= len(order)
                            if ab_no_copy:
                                pass
                            elif copy3:
                                cengs = (nc.vector, nc.scalar, nc.gpsimd)
                                for h in range(2):
                                    eng = cengs[(tt * 2 + h) % 3]
                                    if eng is nc.scalar:
                                        eng.copy(
                                            out=zt[:, g, h * 512 : (h + 1) * 512],
                                            in_=ps[:, h * 512 : (h + 1) * 512],
                                        )
                                    else:
                                        eng.tensor_copy(
                                            out=zt[:, g, h * 512 : (h + 1) * 512],
                                            in_=ps[:, h * 512 : (h + 1) * 512],
                                        )
                            elif tail_dve == 2 and not last_batch:
                                nc.vector.tensor_copy(out=zt[:, g, :512], in_=ps[:, :512])
                                nc.scalar.copy(out=zt[:, g, 512:], in_=ps[:, 512:])
                            elif tt % 2 == 0 or (tail_dve and last_batch):
                                nc.vector.tensor_copy(out=zt[:, g, :], in_=ps[:])
                            else:
                                nc.scalar.copy(out=zt[:, g, :], in_=ps[:])
                    last_batch_s = oi + gb >= len(order)
                    if store_split == 2:
                        seng = nc.scalar if last_batch_s else nc.sync
                    else:
                        seng = nc.scalar if (store_split and (t // gbatch) % 2) else nc.sync
                    if not ab_no_store:
                        seng.dma_start(out=out_d[:, t : t + gb, :], in_=zt[:])
                    for sl in deferred_w:
                        nc.sync.dma_start(out=wcat[:, sl], in_=wcat_d[:, sl])
                    deferred_w = []
                    oi += gb

            if loop_n is None:
                for _ in range(repeat):
                    body()
            else:
                with tc.For_i(0, loop_n, 1) as _i:
                    body()
    nc.compile()
    return nc


def _prep_inputs(embs, ws, plan, mode=MODE):
    wcat = np.zeros((P, NCHUNK * D), _BF16)
    for b in range(4):
        for c in range(KS[b]):
            wcat[:, (WOFF[b] + c) * D : (WOFF[b] + c + 1) * D] = ws[b][
                c * P : (c + 1) * P, :
            ].astype(_BF16)
    ebf = [e.astype(_BF16) for e in embs]
    in_maps = []
    for c in range(NCORES):
        m = {}
        for b in range(4):
            base = int(plan.wbase[b, c])
            w = plan.wrows[b]
            win = ebf[b][base : base + w]
            if win.shape[0] < w:  # window runs past the table end: zero-pad
                win = np.concatenate(
                    [win, np.zeros((w - win.shape[0], DS[b]), _BF16)]
                )
            m[f"e{b}"] = np.ascontiguousarray(win)
        m["wcat"] = wcat
        m["gidx"] = np.ascontiguousarray(plan.gidx[c])
        in_maps.append(m)
    return in_maps


def _assemble(plan, mode, results, repeat=1):
    out = np.empty((NTOK, D), np.float32)
    for c in range(NCORES):
        r = results[c]["out"]  # [128, T, D] partition-major
        r = np.ascontiguousarray(r.transpose(1, 0, 2)).reshape(-1, D)
        valid = plan.rowpos[c] >= 0
        out[plan.rowpos[c][valid]] = r[valid].astype(np.float32)
    return out.reshape(NCORES, SEQ, D)


def run(inputs, mode=MODE, trace=False):
    x = np.asarray(inputs["x"])
    embs = [np.asarray(inputs[f"emb{b}"]) for b in range(4)]
    ws = [np.asarray(inputs[f"W{b}"]) for b in range(4)]
    assert x.shape == (NCORES, SEQ), x.shape

    plan = _plan(x)
    key = (tuple(plan.alloc), tuple(plan.wrows), mode)
    if key not in _cache:
        _cache[key] = _build(plan, mode)
    nc = _cache[key]

    in_maps = _prep_inputs(embs, ws, plan, mode)
    res = run_bass_kernel_spmd(
        nc, in_maps, core_ids=list(range(NCORES)), trace=trace
    )
    out = _assemble(plan, mode, res.results)
    return out, res


def kernel(**inputs):
    out, _ = run(inputs, mode=MODE, trace=False)
    return out



# revision 37
# speedup vs baseline: 1.0800x; 1.0800x over previous
"""Adaptive-input-embedding Bass kernel for one TRN2 chip (8 NeuronCores).

Strategy: token-parallel across the 8 cores — the 32768 tokens are grouped by
bucket, sorted by table index, and dealt as contiguous runs to the cores, so
every core processes ~4096 tokens with identical compile-time structure.
Tables and projection matrices are replicated, except that each core receives
only a <=32k-row *window* of the 237k-row tail-bucket table covering its run
(this keeps gather indices within int16 for the DMA-gather engine).

Device side: per bucket, dma_gather(transpose=True) calls (<=768 indices
each) pull the bf16 embedding rows into SBUF already transposed ([d, tokens]
chunks, i.e. matmul lhsT layout); per 128-token tile the d/128 chunk matmuls
accumulate into PSUM fp32 against the resident bf16 projection chunks; PSUM
is copied to SBUF (alternating DVE/ACT) and written out with large
contiguous partition-major DMA stores (alternating the two HWDGE rings).
The host scatters the returned rows to their token positions while
assembling the full output (the unshard step).
"""

import sys

import numpy as np

try:
    import concourse  # noqa: F401
except ImportError:
    sys.path.insert(0, "/opt/trn_rl_repo")

import ml_dtypes
from concourse import bacc, mybir, tile
from concourse.bass_utils import run_bass_kernel_spmd

BUCKETS = (0, 300, 3000, 30000, 267734)
SIZES = [BUCKETS[i + 1] - BUCKETS[i] for i in range(4)]
D = 1024
DS = [1024, 512, 256, 128]  # embedding dim per bucket
KS = [8, 4, 2, 1]  # 128-chunks per bucket
WOFF = [0, 8, 12, 14]  # chunk offset of each bucket in wcat
NCHUNK = 15
SUB = 32768  # rows addressable by one int16 gather call
NCORES = 8
SEQ = 4096
NTOK = NCORES * SEQ
P = 128
GB = 8  # tiles per store batch

MODE = "seq_bf16"

_BF16 = ml_dtypes.bfloat16

_cache: dict = {}


def _r16(v):
    return -(-int(v) // 16) * 16


def _r128(v):
    return -(-int(v) // 128) * 128


class Plan:
    pass


def _plan(x):
    """Global bucketing + even dealing of each bucket across the cores.

    Bucket 3 (237k rows) is dealt as contiguous runs of the index-sorted
    token list, so each core's gather indices span < 32k table rows and fit
    int16 against a per-core window of the table (passed as that core's e3
    input). Produces identical compile-time structure for all cores."""
    xf = x.reshape(-1).astype(np.int64)
    assert xf.shape[0] == NTOK
    bkt = np.searchsorted(np.asarray(BUCKETS), xf, side="right") - 1
    bkt = np.clip(bkt, 0, 3)
    loc = xf - np.asarray(BUCKETS)[bkt]

    # per-(bucket, core) token positions: sort by table index, deal
    # contiguous runs (counts differ by <=1, spans stay narrow for bucket 3)
    per_core_pos = {}
    wbase = np.zeros((4, NCORES), np.int64)  # per-core table window base
    alloc = [0] * 4
    wrows = [0] * 4  # table window rows (compile-time shape)
    for b in range(4):
        pos = np.nonzero(bkt == b)[0]
        pos = pos[np.argsort(loc[pos], kind="stable")]
        n = pos.size
        cnt = np.full(NCORES, n // NCORES)
        cnt[: n % NCORES] += 1
        cuts = np.concatenate([[0], np.cumsum(cnt)])

        def spans(cuts_):
            sp, mx = 0, 0
            for c in range(NCORES):
                pc = pos[cuts_[c] : cuts_[c + 1]]
                if pc.size:
                    sp = max(sp, int(loc[pc[-1]] - loc[pc[0]]) + 1)
                    mx = max(mx, pc.size)
            return sp, mx

        span, mxc = spans(cuts)
        if b == 3 and span > SUB:
            # skewed distribution: balanced cuts straddle >32k-row ranges;
            # fall back to fixed 32k-row boundary cuts (unbalanced counts
            # but indices stay int16 against each core's window)
            edges = np.searchsorted(loc[pos], np.arange(1, NCORES) * SUB)
            cuts = np.concatenate([[0], edges, [n]])
            span, mxc = spans(cuts)
        for c in range(NCORES):
            pc = pos[cuts[c] : cuts[c + 1]]
            per_core_pos[(b, c)] = pc
            if pc.size:
                wbase[b, c] = loc[pc[0]]
        alloc[b] = int(_r16(mxc))
        wrows[b] = min(span if b == 3 else SIZES[b], SIZES[b])
        wrows[b] = max(wrows[b], 1)
        assert wrows[b] <= SUB, (b, wrows[b])
        if b < 3:
            wbase[b] = 0

    # slot layout: one 128-aligned block per bucket
    segs = []  # (bucket, o_slot, n_alloc, num_idxs)
    blocks = []
    o = 0
    for b in range(4):
        ni = _r128(alloc[b])
        segs.append((b, o, alloc[b], ni))
        blocks.append((o, ni))
        o += ni
    ntot = o

    p = Plan()
    p.segs, p.blocks, p.ntot = segs, blocks, ntot
    p.t_total = ntot // P
    p.alloc = alloc
    p.wrows = wrows
    p.wbase = wbase

    gidx = np.zeros((NCORES, P, ntot // 16), np.int16)
    rowpos = np.full((NCORES, ntot), -1, np.int64)  # slot -> global token pos
    for b, o, na, ni in segs:
        for c in range(NCORES):
            pos = per_core_pos[(b, c)]
            n = pos.size
            li = np.zeros(na, np.int64)
            li[:n] = loc[pos] - wbase[b, c]
            rowpos[c, o : o + n] = pos
            ii = np.arange(na)
            cols = o // 16 + ii // 16
            rows = ii % 16
            for g in range(8):  # replicate across the 8 groups of 16 partitions
                gidx[c, g * 16 + rows, cols] = li.astype(np.int16)
    p.gidx, p.rowpos = gidx, rowpos
    return p


def _build(plan, mode=MODE, repeat=1, loop_n=None, b3_first=True, split_w=True, gbatch=16, zbufs=5, psbufs=4, store_split=True, tail_dve=True, ab_no_store=False, ab_no_gather=False, ab_no_mm=False, ab_no_copy=False, g_transpose=True, g_cap=768, g_sp=True, g_q=4, g_head=128, ps_half=False, tail_split=0, ab_same_w=False, copy3=False, gsched=(1, 1, 2, 4, 8, 8, 8, 8, 8, 8), unroll=1, gbufs=2):
    """Build + compile the SPMD Bass program.

    repeat>1 re-emits the whole body; loop_n wraps the body in a HW For_i
    loop (both used only for differential timing)."""
    ntot, t_total = plan.ntot, plan.t_total
    bf16 = mybir.dt.bfloat16
    f32 = mybir.dt.float32
    odt = bf16 if mode.endswith("bf16") else f32

    assert 8 % g_q == 0, "g_q must divide the 8 DMASW sem lanes"
    nc = bacc.Bacc(None, target_bir_lowering=False, num_swdge_queues=g_q)
    # Pool-engine DMA instructions take DMASW sem lanes round-robin (8 lanes);
    # the sim (and sane HW sync) requires each lane to stay on ONE SWDGE
    # queue, so derive each gather's queue from the lane it will be assigned.
    pool_dma_n = [0]

    def _pool_q():
        q = (pool_dma_n[0] % 8) % g_q
        pool_dma_n[0] += 1
        return q
    e_d = [
        nc.declare_dram_parameter(f"e{b}", [plan.wrows[b], DS[b]], bf16, isOutput=False)
        for b in range(4)
    ]
    wcat_d = nc.declare_dram_parameter("wcat", [P, NCHUNK * D], bf16, isOutput=False)
    gidx_d = nc.declare_dram_parameter("gidx", [P, ntot // 16], mybir.dt.int16, isOutput=False)
    # partition-major: slot s lives at out[s % 128, s // 128, :] so each
    # partition's store stream is contiguous (few, large descriptors)
    out_d = nc.declare_dram_parameter("out", [P, t_total, D], odt, isOutput=True)

    bbase = [blk[0] for blk in plan.blocks]
    bslots = [blk[1] for blk in plan.blocks]

    with tile.TileContext(nc) as tc:
        with (
            tc.tile_pool(name="const", bufs=1) as cp,
            tc.tile_pool(name="gbuf", bufs=gbufs) as gp,
            tc.tile_pool(name="zbuf", bufs=zbufs) as zp,
            tc.tile_pool(name="ps", bufs=psbufs, space="PSUM") as pp,
        ):
            gidx = cp.tile([P, ntot // 16], mybir.dt.int16)
            nc.gpsimd.dma_start(out=gidx[:], in_=gidx_d[:])
            _pool_q()  # gidx load consumes a DMASW lane
            wcat = cp.tile([P, NCHUNK * D], bf16)
            if split_w:
                # W3 first: bucket-3 tiles are processed first and its W slice
                # is small, so the first matmuls aren't gated on the full load
                upfront = (3,) if split_w == 3 else (3, 2, 1, 0)
                for b in upfront:
                    sl = slice(WOFF[b] * D, (WOFF[b] + KS[b]) * D)
                    eng = nc.sync if (split_w is True or split_w in (1, 3) or b >= 2) else nc.scalar
                    eng.dma_start(out=wcat[:, sl], in_=wcat_d[:, sl])
            else:
                nc.sync.dma_start(out=wcat[:], in_=wcat_d[:])



            def bucket_of_tile(t):
                slot = t * P
                for b in range(4):
                    if bbase[b] <= slot < bbase[b] + bslots[b]:
                        return b
                raise AssertionError(t)

            def body(_iv=None):
                gt = [
                    gp.tile([P, KS[b], bslots[b]], bf16, name=f"G{b}", tag=f"G{b}")
                    if bslots[b]
                    else None
                    for b in range(4)
                ]
                if ab_no_gather and not ab_no_mm:
                    for b in range(4):  # dummy-allocate gt for the ablation
                        nc.vector.memset(gt[b][:, :, :1], 0)
                deferred_w = []
                if split_w == 3:
                    for b in (2, 1, 0):
                        sl = slice(WOFF[b] * D, (WOFF[b] + KS[b]) * D)
                        deferred_w.append(sl)
                GCAP = g_cap  # >=1024 idxs in one SWDGE gather wedges the device
                gcall = 0
                gsegs = (
                    sorted(plan.segs, key=lambda e: e[0] != 3)
                    if b3_first
                    else plan.segs
                )

                def chunks_for(b, ni, first):
                    if KS[b] != 1:
                        return [(0, ni)]
                    out, k = [], 0
                    if first and g_head:
                        # one small head call per SWDGE queue so the first
                        # tiles land fast on all queues concurrently
                        for _ in range(g_q):
                            if k + g_head > ni:
                                break
                            out.append((k, g_head))
                            k += g_head
                    while k < ni:
                        nk = min(GCAP, ni - k)
                        out.append((k, nk))
                        k += nk
                    return out

                for si, (b, o, na, ni) in enumerate(gsegs):
                    if ab_no_gather:
                        break
                    for k, nk in chunks_for(b, ni, si == 0):
                        ok = o + k
                        o_local = ok - bbase[b]
                        if g_transpose:
                            out_ap = gt[b][:, :, o_local : o_local + nk]
                        else:
                            gnt = gp.tile(
                                [P, nk // P, DS[b]],
                                bf16,
                                name=f"GN{gcall}",
                                tag=f"GN{gcall}",
                            )
                            out_ap = gnt[:]
                        nc.gpsimd.dma_gather(
                            out_ap=out_ap,
                            in_ap=e_d[b][:],
                            idxs_ap=gidx[:, ok // 16 : ok // 16 + nk // 16],
                            num_idxs=nk,
                            num_idxs_reg=nk,
                            elem_size=DS[b],
                            transpose=g_transpose,
                            single_packet=g_sp,
                            queue_num=_pool_q(),
                        )
                        gcall += 1

                # bucket-3 tiles first (largest block, cheapest W), then 0..2
                t3 = bbase[3] // P
                if b3_first:
                    order = list(range(t3, t_total)) + list(range(0, t3))
                else:
                    order = list(range(t_total))
                oi = 0
                batch_i = 0
                while oi < len(order):
                    t = order[oi]
                    rem = len(order) - oi
                    if gsched is not None:
                        eff_gb = gsched[batch_i] if batch_i < len(gsched) else gsched[-1]
                    else:
                        eff_gb = 2 if (tail_split and rem <= tail_split) else gbatch
                    batch_i += 1
                    gb = 1
                    for g in range(1, min(eff_gb, rem)):
                        if order[oi + g] == t + g:
                            gb += 1
                        else:
                            break
                    zt = zp.tile([P, gb, D], odt, tag="z")
                    if ab_no_mm or ab_no_copy:
                        nc.vector.memset(zt[:, :, :1], 0)
                    for g in range(gb if not ab_no_mm else 0):
                        tt = t + g
                        b = bucket_of_tile(tt)
                        ts0 = tt * P - bbase[b]
                        kb = KS[b]
                        if ps_half:
                            ps0 = pp.tile([P, 512], f32, tag="ps0")
                            ps1 = pp.tile([P, 512], f32, tag="ps1")
                            pss = (ps0, ps1)
                            for c in range(kb):
                                lhsT = gt[b][:, c, ts0 : ts0 + P]
                                for h in range(2):
                                    nc.tensor.matmul(
                                        out=pss[h][:, :],
                                        lhsT=lhsT,
                                        rhs=wcat[:, (WOFF[b] + c) * D + h * 512 :][:, :512],
                                        start=(c == 0),
                                        stop=(c == kb - 1),
                                    )
                            if not ab_no_copy:
                                nc.vector.tensor_copy(out=zt[:, g, :512], in_=ps0[:])
                                nc.scalar.copy(out=zt[:, g, 512:], in_=ps1[:])
                            oi_done = True
                        else:
                            ps = pp.tile([P, D], f32, tag="ps")
                            for c in range(kb):
                                lhsT = (
                                    gt[3][:, 0, 0:P]
                                    if ab_same_w
                                    else gt[b][:, c, ts0 : ts0 + P]
                                )
                                for h in range(2):
                                    nc.tensor.matmul(
                                        out=ps[:, h * 512 : (h + 1) * 512],
                                        lhsT=lhsT,
                                        rhs=wcat[:, (WOFF[b] + c) * D + h * 512 :][:, :512],
                                        start=(c == 0),
                                        stop=(c == kb - 1),
                                    )
                            last_batch = oi + gb >= len(order)
                            if ab_no_copy:
                                pass
                            elif copy3:
                                cengs = (nc.vector, nc.scalar, nc.gpsimd)
                                for h in range(2):
                                    eng = cengs[(tt * 2 + h) % 3]
                                    if eng is nc.scalar:
                                        eng.copy(
                                            out=zt[:, g, h * 512 : (h + 1) * 512],
                                            in_=ps[:, h * 512 : (h + 1) * 512],
                                        )
                                    else:
                                        eng.tensor_copy(
                                            out=zt[:, g, h * 512 : (h + 1) * 512],
                                            in_=ps[:, h * 512 : (h + 1) * 512],
                                        )
                            elif tail_dve == 2 and not last_batch:
                                nc.vector.tensor_copy(out=zt[:, g, :512], in_=ps[:, :512])
                                nc.scalar.copy(out=zt[:, g, 512:], in_=ps[:, 512:])
                            elif tt % 2 == 0 or (tail_dve and last_batch):
                                nc.vector.tensor_copy(out=zt[:, g, :], in_=ps[:])
                            else:
                                nc.scalar.copy(out=zt[:, g, :], in_=ps[:])
                    last_batch_s = oi + gb >= len(order)
                    if store_split == 2:
                        seng = nc.scalar if last_batch_s else nc.sync
                    else:
                        seng = nc.scalar if (store_split and (t // gbatch) % 2) else nc.sync
                    if not ab_no_store:
                        seng.dma_start(out=out_d[:, t : t + gb, :], in_=zt[:])
                    for sl in deferred_w:
                        nc.sync.dma_start(out=wcat[:, sl], in_=wcat_d[:, sl])
                    deferred_w = []
                    oi += gb

            if loop_n is None:
                for _ in range(repeat):
                    body()
            else:
                with tc.For_i(0, loop_n, 1) as _i:
                    for _ in range(unroll):
                        body()
    nc.compile()
    return nc


def _prep_inputs(embs, ws, plan, mode=MODE):
    wcat = np.zeros((P, NCHUNK * D), _BF16)
    for b in range(4):
        for c in range(KS[b]):
            wcat[:, (WOFF[b] + c) * D : (WOFF[b] + c + 1) * D] = ws[b][
                c * P : (c + 1) * P, :
            ].astype(_BF16)
    ebf = [e.astype(_BF16) for e in embs]
    in_maps = []
    for c in range(NCORES):
        m = {}
        for b in range(4):
            base = int(plan.wbase[b, c])
            w = plan.wrows[b]
            win = ebf[b][base : base + w]
            if win.shape[0] < w:  # window runs past the table end: zero-pad
                win = np.concatenate(
                    [win, np.zeros((w - win.shape[0], DS[b]), _BF16)]
                )
            m[f"e{b}"] = np.ascontiguousarray(win)
        m["wcat"] = wcat
        m["gidx"] = np.ascontiguousarray(plan.gidx[c])
        in_maps.append(m)
    return in_maps


def _assemble(plan, mode, results, repeat=1):
    out = np.empty((NTOK, D), np.float32)
    for c in range(NCORES):
        r = results[c]["out"]  # [128, T, D] partition-major
        r = np.ascontiguousarray(r.transpose(1, 0, 2)).reshape(-1, D)
        valid = plan.rowpos[c] >= 0
        out[plan.rowpos[c][valid]] = r[valid].astype(np.float32)
    return out.reshape(NCORES, SEQ, D)


def run(inputs, mode=MODE, trace=False):
    x = np.asarray(inputs["x"])
    embs = [np.asarray(inputs[f"emb{b}"]) for b in range(4)]
    ws = [np.asarray(inputs[f"W{b}"]) for b in range(4)]
    assert x.shape == (NCORES, SEQ), x.shape

    plan = _plan(x)
    key = (tuple(plan.alloc), tuple(plan.wrows), mode)
    if key not in _cache:
        _cache[key] = _build(plan, mode)
    nc = _cache[key]

    in_maps = _prep_inputs(embs, ws, plan, mode)
    res = run_bass_kernel_spmd(
        nc, in_maps, core_ids=list(range(NCORES)), trace=trace
    )
    out = _assemble(plan, mode, res.results)
    return out, res


def kernel(**inputs):
    out, _ = run(inputs, mode=MODE, trace=False)
    return out



# revision 46
# speedup vs baseline: 1.1920x; 1.1037x over previous
"""Adaptive-input-embedding Bass kernel for one TRN2 chip (8 NeuronCores).

Strategy: token-parallel across the 8 cores — the 32768 tokens are grouped by
bucket, sorted by table index, and dealt as contiguous runs to the cores, so
every core processes ~4096 tokens with identical compile-time structure.
Tables and projection matrices are replicated, except that each core receives
only a <=32k-row *window* of the 237k-row tail-bucket table covering its run
(this keeps gather indices within int16 for the DMA-gather engine).

Device side: per bucket, dma_gather(transpose=True) calls (<=768 indices
each) pull the bf16 embedding rows into SBUF already transposed ([d, tokens]
chunks, i.e. matmul lhsT layout); per 128-token tile the d/128 chunk matmuls
accumulate into PSUM fp32 against the resident bf16 projection chunks; PSUM
is copied to SBUF (alternating DVE/ACT) and written out with large
contiguous partition-major DMA stores (alternating the two HWDGE rings).
The host scatters the returned rows to their token positions while
assembling the full output (the unshard step).
"""

import sys

import numpy as np

try:
    import concourse  # noqa: F401
except ImportError:
    sys.path.insert(0, "/opt/trn_rl_repo")

import ml_dtypes
from concourse import bacc, mybir, tile
from concourse.bass_utils import run_bass_kernel_spmd

BUCKETS = (0, 300, 3000, 30000, 267734)
SIZES = [BUCKETS[i + 1] - BUCKETS[i] for i in range(4)]
D = 1024
DS = [1024, 512, 256, 128]  # embedding dim per bucket
KS = [8, 4, 2, 1]  # 128-chunks per bucket
WOFF = [0, 8, 12, 14]  # chunk offset of each bucket in wcat
NCHUNK = 15
SUB = 32768  # rows addressable by one int16 gather call
NCORES = 8
SEQ = 4096
NTOK = NCORES * SEQ
P = 128
GB = 8  # tiles per store batch

MODE = "seq_bf16"

_BF16 = ml_dtypes.bfloat16

_cache: dict = {}


def _r16(v):
    return -(-int(v) // 16) * 16


def _r128(v):
    return -(-int(v) // 128) * 128


class Plan:
    pass


def _plan(x):
    """Global bucketing + even dealing of each bucket across the cores.

    Bucket 3 (237k rows) is dealt as contiguous runs of the index-sorted
    token list, so each core's gather indices span < 32k table rows and fit
    int16 against a per-core window of the table (passed as that core's e3
    input). Produces identical compile-time structure for all cores."""
    xf = x.reshape(-1).astype(np.int64)
    assert xf.shape[0] == NTOK
    bkt = np.searchsorted(np.asarray(BUCKETS), xf, side="right") - 1
    bkt = np.clip(bkt, 0, 3)
    loc = xf - np.asarray(BUCKETS)[bkt]

    # per-(bucket, core) token positions: sort by table index, deal
    # contiguous runs (counts differ by <=1, spans stay narrow for bucket 3)
    per_core_pos = {}
    wbase = np.zeros((4, NCORES), np.int64)  # per-core table window base
    alloc = [0] * 4
    wrows = [0] * 4  # table window rows (compile-time shape)
    for b in range(4):
        pos = np.nonzero(bkt == b)[0]
        pos = pos[np.argsort(loc[pos], kind="stable")]
        n = pos.size
        cnt = np.full(NCORES, n // NCORES)
        cnt[: n % NCORES] += 1
        cuts = np.concatenate([[0], np.cumsum(cnt)])

        def spans(cuts_):
            sp, mx = 0, 0
            for c in range(NCORES):
                pc = pos[cuts_[c] : cuts_[c + 1]]
                if pc.size:
                    sp = max(sp, int(loc[pc[-1]] - loc[pc[0]]) + 1)
                    mx = max(mx, pc.size)
            return sp, mx

        span, mxc = spans(cuts)
        if b == 3 and span > SUB:
            # skewed distribution: balanced cuts straddle >32k-row ranges;
            # fall back to fixed 32k-row boundary cuts (unbalanced counts
            # but indices stay int16 against each core's window)
            edges = np.searchsorted(loc[pos], np.arange(1, NCORES) * SUB)
            cuts = np.concatenate([[0], edges, [n]])
            span, mxc = spans(cuts)
        for c in range(NCORES):
            pc = pos[cuts[c] : cuts[c + 1]]
            per_core_pos[(b, c)] = pc
            if pc.size:
                wbase[b, c] = loc[pc[0]]
        alloc[b] = int(_r16(mxc))
        wrows[b] = min(span if b == 3 else SIZES[b], SIZES[b])
        wrows[b] = max(wrows[b], 1)
        assert wrows[b] <= SUB, (b, wrows[b])
        if b < 3:
            wbase[b] = 0

    # gather layout: one 128-aligned block per bucket (gidx/gt addressing)
    segs = []  # (bucket, o_gather, n_alloc, num_idxs)
    blocks = []
    o = 0
    for b in range(4):
        ni = _r128(alloc[b])
        segs.append((b, o, alloc[b], ni))
        blocks.append((o, ni))
        o += ni
    ntot = o

    # slot (output) layout: 64-aligned packed blocks, bucket 3 first, so at
    # most one tile boundary mixes two buckets and pad tiles are eliminated
    slot_blocks = []  # (bucket, slot_base, slot_size) in processing order
    so = 0
    for b in (3, 2, 1, 0):
        sz = -(-alloc[b] // 64) * 64
        slot_blocks.append((b, so, sz))
        so += sz
    ntot_s = _r128(so)

    p = Plan()
    p.segs, p.blocks, p.ntot = segs, blocks, ntot
    p.t_total = ntot // P
    p.slot_blocks, p.ntot_s = slot_blocks, ntot_s
    p.t_total_s = ntot_s // P
    p.alloc = alloc
    p.wrows = wrows
    p.wbase = wbase

    gidx = np.zeros((NCORES, P, ntot // 16), np.int16)
    rowpos = np.full((NCORES, ntot), -1, np.int64)  # gslot -> global token pos
    rowpos_s = np.full((NCORES, ntot_s), -1, np.int64)  # packed slot -> pos
    sbase = {b: sb for b, sb, _ in slot_blocks}
    for b, o, na, ni in segs:
        for c in range(NCORES):
            pos = per_core_pos[(b, c)]
            n = pos.size
            li = np.zeros(na, np.int64)
            li[:n] = loc[pos] - wbase[b, c]
            rowpos[c, o : o + n] = pos
            rowpos_s[c, sbase[b] : sbase[b] + n] = pos
            ii = np.arange(na)
            cols = o // 16 + ii // 16
            rows = ii % 16
            for g in range(8):  # replicate across the 8 groups of 16 partitions
                gidx[c, g * 16 + rows, cols] = li.astype(np.int16)
    p.gidx, p.rowpos, p.rowpos_s = gidx, rowpos, rowpos_s
    return p


def _build(plan, mode=MODE, repeat=1, loop_n=None, b3_first=True, split_w=True, gbatch=16, zbufs=5, psbufs=4, store_split=True, tail_dve=True, ab_no_store=False, ab_no_gather=False, ab_no_mm=False, ab_no_copy=False, g_transpose=True, g_cap=768, g_sp=True, g_q=4, g_head=128, ps_half=False, tail_split=0, ab_same_w=False, copy3=False, gsched=(1, 1, 2, 4, 8, 8, 8, 8, 4, 2), unroll=1, gbufs=2, pack64=True):
    """Build + compile the SPMD Bass program.

    repeat>1 re-emits the whole body; loop_n wraps the body in a HW For_i
    loop (both used only for differential timing)."""
    ntot, t_total = plan.ntot, plan.t_total
    bf16 = mybir.dt.bfloat16
    f32 = mybir.dt.float32
    odt = bf16 if mode.endswith("bf16") else f32

    assert 8 % g_q == 0, "g_q must divide the 8 DMASW sem lanes"
    nc = bacc.Bacc(None, target_bir_lowering=False, num_swdge_queues=g_q)
    # Pool-engine DMA instructions take DMASW sem lanes round-robin (8 lanes);
    # the sim (and sane HW sync) requires each lane to stay on ONE SWDGE
    # queue, so derive each gather's queue from the lane it will be assigned.
    pool_dma_n = [0]

    def _pool_q():
        q = (pool_dma_n[0] % 8) % g_q
        pool_dma_n[0] += 1
        return q
    e_d = [
        nc.declare_dram_parameter(f"e{b}", [plan.wrows[b], DS[b]], bf16, isOutput=False)
        for b in range(4)
    ]
    wcat_d = nc.declare_dram_parameter("wcat", [P, NCHUNK * D], bf16, isOutput=False)
    gidx_d = nc.declare_dram_parameter("gidx", [P, ntot // 16], mybir.dt.int16, isOutput=False)
    # partition-major: slot s lives at out[s % 128, s // 128, :] so each
    # partition's store stream is contiguous (few, large descriptors)
    t_out = plan.t_total_s if pack64 else t_total
    out_d = nc.declare_dram_parameter("out", [P, t_out, D], odt, isOutput=True)

    bbase = [blk[0] for blk in plan.blocks]
    bslots = [blk[1] for blk in plan.blocks]

    with tile.TileContext(nc) as tc:
        with (
            tc.tile_pool(name="const", bufs=1) as cp,
            tc.tile_pool(name="gbuf", bufs=gbufs) as gp,
            tc.tile_pool(name="zbuf", bufs=zbufs) as zp,
            tc.tile_pool(name="ps", bufs=psbufs, space="PSUM") as pp,
        ):
            gidx = cp.tile([P, ntot // 16], mybir.dt.int16)
            nc.gpsimd.dma_start(out=gidx[:], in_=gidx_d[:])
            _pool_q()  # gidx load consumes a DMASW lane
            wcat = cp.tile([P, NCHUNK * D], bf16)
            if split_w:
                # W3 first: bucket-3 tiles are processed first and its W slice
                # is small, so the first matmuls aren't gated on the full load
                upfront = (3,) if split_w == 3 else (3, 2, 1, 0)
                for b in upfront:
                    sl = slice(WOFF[b] * D, (WOFF[b] + KS[b]) * D)
                    eng = nc.sync if (split_w is True or split_w in (1, 3) or b >= 2) else nc.scalar
                    eng.dma_start(out=wcat[:, sl], in_=wcat_d[:, sl])
            else:
                nc.sync.dma_start(out=wcat[:], in_=wcat_d[:])



            def bucket_of_tile(t):
                slot = t * P
                for b in range(4):
                    if bbase[b] <= slot < bbase[b] + bslots[b]:
                        return b
                raise AssertionError(t)

            def segments_of_tile(t):
                # (bucket, in-bucket offset, psum partition offset, ncols)
                if not pack64:
                    b = bucket_of_tile(t)
                    return [(b, t * P - bbase[b], 0, P)]
                out = []
                for b, sb, sz in plan.slot_blocks:
                    lo = max(t * P, sb)
                    hi = min((t + 1) * P, sb + sz)
                    if lo < hi:
                        out.append((b, lo - sb, lo - t * P, hi - lo))
                return out

            def body(_iv=None):
                gt = [
                    gp.tile([P, KS[b], bslots[b]], bf16, name=f"G{b}", tag=f"G{b}")
                    if bslots[b]
                    else None
                    for b in range(4)
                ]
                if ab_no_gather and not ab_no_mm:
                    for b in range(4):  # dummy-allocate gt for the ablation
                        nc.vector.memset(gt[b][:, :, :1], 0)
                deferred_w = []
                if split_w == 3:
                    for b in (2, 1, 0):
                        sl = slice(WOFF[b] * D, (WOFF[b] + KS[b]) * D)
                        deferred_w.append(sl)
                GCAP = g_cap  # >=1024 idxs in one SWDGE gather wedges the device
                gcall = 0
                gsegs = (
                    sorted(plan.segs, key=lambda e: e[0] != 3)
                    if b3_first
                    else plan.segs
                )

                def chunks_for(b, ni, first):
                    if KS[b] != 1:
                        return [(0, ni)]
                    out, k = [], 0
                    if first and g_head:
                        # one small head call per SWDGE queue so the first
                        # tiles land fast on all queues concurrently
                        for _ in range(g_q):
                            if k + g_head > ni:
                                break
                            out.append((k, g_head))
                            k += g_head
                    while k < ni:
                        nk = min(GCAP, ni - k)
                        out.append((k, nk))
                        k += nk
                    return out

                for si, (b, o, na, ni) in enumerate(gsegs):
                    if ab_no_gather:
                        break
                    for k, nk in chunks_for(b, ni, si == 0):
                        ok = o + k
                        o_local = ok - bbase[b]
                        if g_transpose:
                            out_ap = gt[b][:, :, o_local : o_local + nk]
                        else:
                            gnt = gp.tile(
                                [P, nk // P, DS[b]],
                                bf16,
                                name=f"GN{gcall}",
                                tag=f"GN{gcall}",
                            )
                            out_ap = gnt[:]
                        nc.gpsimd.dma_gather(
                            out_ap=out_ap,
                            in_ap=e_d[b][:],
                            idxs_ap=gidx[:, ok // 16 : ok // 16 + nk // 16],
                            num_idxs=nk,
                            num_idxs_reg=nk,
                            elem_size=DS[b],
                            transpose=g_transpose,
                            single_packet=g_sp,
                            queue_num=_pool_q(),
                        )
                        gcall += 1

                # bucket-3 tiles first (largest block, cheapest W), then 0..2
                if pack64:
                    order = list(range(plan.t_total_s))  # b3-first by layout
                else:
                    t3 = bbase[3] // P
                    if b3_first:
                        order = list(range(t3, t_total)) + list(range(0, t3))
                    else:
                        order = list(range(t_total))
                oi = 0
                batch_i = 0
                while oi < len(order):
                    t = order[oi]
                    rem = len(order) - oi
                    if gsched is not None:
                        eff_gb = gsched[batch_i] if batch_i < len(gsched) else gsched[-1]
                    else:
                        eff_gb = 2 if (tail_split and rem <= tail_split) else gbatch
                    batch_i += 1
                    gb = 1
                    for g in range(1, min(eff_gb, rem)):
                        if order[oi + g] == t + g:
                            gb += 1
                        else:
                            break
                    zt = zp.tile([P, gb, D], odt, tag="z")
                    if ab_no_mm or ab_no_copy:
                        nc.vector.memset(zt[:, :, :1], 0)
                    for g in range(gb if not ab_no_mm else 0):
                        tt = t + g
                        if not pack64:
                            b = bucket_of_tile(tt)
                            ts0 = tt * P - bbase[b]
                            kb = KS[b]
                        else:
                            assert not ps_half
                        if ps_half:
                            ps0 = pp.tile([P, 512], f32, tag="ps0")
                            ps1 = pp.tile([P, 512], f32, tag="ps1")
                            pss = (ps0, ps1)
                            for c in range(kb):
                                lhsT = gt[b][:, c, ts0 : ts0 + P]
                                for h in range(2):
                                    nc.tensor.matmul(
                                        out=pss[h][:, :],
                                        lhsT=lhsT,
                                        rhs=wcat[:, (WOFF[b] + c) * D + h * 512 :][:, :512],
                                        start=(c == 0),
                                        stop=(c == kb - 1),
                                    )
                            if not ab_no_copy:
                                nc.vector.tensor_copy(out=zt[:, g, :512], in_=ps0[:])
                                nc.scalar.copy(out=zt[:, g, 512:], in_=ps1[:])
                            oi_done = True
                        else:
                            ps = pp.tile([P, D], f32, tag="ps")
                            for b2_, boff, m0, ncols in segments_of_tile(tt):
                                kb2 = KS[b2_]
                                for c in range(kb2):
                                    lhsT = (
                                        gt[3][:, 0, 0:P]
                                        if ab_same_w
                                        else gt[b2_][:, c, boff : boff + ncols]
                                    )
                                    for h in range(2):
                                        nc.tensor.matmul(
                                            out=ps[m0 : m0 + ncols, h * 512 : (h + 1) * 512],
                                            lhsT=lhsT,
                                            rhs=wcat[:, (WOFF[b2_] + c) * D + h * 512 :][:, :512],
                                            start=(c == 0),
                                            stop=(c == kb2 - 1),
                                        )
                            last_batch = oi + gb >= len(order)
                            if ab_no_copy:
                                pass
                            elif copy3:
                                cengs = (nc.vector, nc.scalar, nc.gpsimd)
                                for h in range(2):
                                    eng = cengs[(tt * 2 + h) % 3]
                                    if eng is nc.scalar:
                                        eng.copy(
                                            out=zt[:, g, h * 512 : (h + 1) * 512],
                                            in_=ps[:, h * 512 : (h + 1) * 512],
                                        )
                                    else:
                                        eng.tensor_copy(
                                            out=zt[:, g, h * 512 : (h + 1) * 512],
                                            in_=ps[:, h * 512 : (h + 1) * 512],
                                        )
                            elif tail_dve == 2 and not last_batch:
                                nc.vector.tensor_copy(out=zt[:, g, :512], in_=ps[:, :512])
                                nc.scalar.copy(out=zt[:, g, 512:], in_=ps[:, 512:])
                            elif tt % 2 == 0 or (tail_dve and last_batch):
                                nc.vector.tensor_copy(out=zt[:, g, :], in_=ps[:])
                            else:
                                nc.scalar.copy(out=zt[:, g, :], in_=ps[:])
                    last_batch_s = oi + gb >= len(order)
                    if store_split == 2:
                        seng = nc.scalar if last_batch_s else nc.sync
                    else:
                        seng = nc.scalar if (store_split and (t // gbatch) % 2) else nc.sync
                    if not ab_no_store:
                        seng.dma_start(out=out_d[:, t : t + gb, :], in_=zt[:])
                    for sl in deferred_w:
                        nc.sync.dma_start(out=wcat[:, sl], in_=wcat_d[:, sl])
                    deferred_w = []
                    oi += gb

            if loop_n is None:
                for _ in range(repeat):
                    body()
            else:
                with tc.For_i(0, loop_n, 1) as _i:
                    for _ in range(unroll):
                        body()
    nc.compile()
    return nc


def _prep_inputs(embs, ws, plan, mode=MODE):
    wcat = np.zeros((P, NCHUNK * D), _BF16)
    for b in range(4):
        for c in range(KS[b]):
            wcat[:, (WOFF[b] + c) * D : (WOFF[b] + c + 1) * D] = ws[b][
                c * P : (c + 1) * P, :
            ].astype(_BF16)
    ebf = [e.astype(_BF16) for e in embs]
    in_maps = []
    for c in range(NCORES):
        m = {}
        for b in range(4):
            base = int(plan.wbase[b, c])
            w = plan.wrows[b]
            win = ebf[b][base : base + w]
            if win.shape[0] < w:  # window runs past the table end: zero-pad
                win = np.concatenate(
                    [win, np.zeros((w - win.shape[0], DS[b]), _BF16)]
                )
            m[f"e{b}"] = np.ascontiguousarray(win)
        m["wcat"] = wcat
        m["gidx"] = np.ascontiguousarray(plan.gidx[c])
        in_maps.append(m)
    return in_maps


def _assemble(plan, mode, results, repeat=1):
    out = np.empty((NTOK, D), np.float32)
    for c in range(NCORES):
        r = results[c]["out"]  # [128, T, D] partition-major
        packed = r.shape[1] == plan.t_total_s
        rp = plan.rowpos_s if packed else plan.rowpos
        r = np.ascontiguousarray(r.transpose(1, 0, 2)).reshape(-1, D)
        valid = rp[c] >= 0
        out[rp[c][valid]] = r[valid].astype(np.float32)
    return out.reshape(NCORES, SEQ, D)


def run(inputs, mode=MODE, trace=False):
    x = np.asarray(inputs["x"])
    embs = [np.asarray(inputs[f"emb{b}"]) for b in range(4)]
    ws = [np.asarray(inputs[f"W{b}"]) for b in range(4)]
    assert x.shape == (NCORES, SEQ), x.shape

    plan = _plan(x)
    key = (tuple(plan.alloc), tuple(plan.wrows), mode)
    if key not in _cache:
        _cache[key] = _build(plan, mode)
    nc = _cache[key]

    in_maps = _prep_inputs(embs, ws, plan, mode)
    res = run_bass_kernel_spmd(
        nc, in_maps, core_ids=list(range(NCORES)), trace=trace
    )
    out = _assemble(plan, mode, res.results)
    return out, res


def kernel(**inputs):
    out, _ = run(inputs, mode=MODE, trace=False)
    return out



# revision 47
# speedup vs baseline: 1.1949x; 1.0024x over previous
"""Adaptive-input-embedding Bass kernel for one TRN2 chip (8 NeuronCores).

Strategy: token-parallel across the 8 cores — the 32768 tokens are grouped by
bucket, sorted by table index, and dealt as contiguous runs to the cores, so
every core processes ~4096 tokens with identical compile-time structure.
Tables and projection matrices are replicated, except that each core receives
only a <=32k-row *window* of the 237k-row tail-bucket table covering its run
(this keeps gather indices within int16 for the DMA-gather engine).

Device side: per bucket, dma_gather(transpose=True) calls (<=768 indices
each) pull the bf16 embedding rows into SBUF already transposed ([d, tokens]
chunks, i.e. matmul lhsT layout). Gathers are spread over 4 SWDGE queues
(queue chosen to match the DMASW sem-lane rotation so each lane stays on one
queue) — a single queue serializes per-call completion and is ~2.5x slower.
The first four gather calls are small (128 idx, one per queue) so the first
tiles land fast and the PE/copy/store pipeline starts early. Output slots
are packed at 64-slot granularity (bucket 3 first, mixed tiles issue
per-segment matmuls at PSUM partition offsets 0/64), eliminating the
per-bucket padding tiles. Per 128-token tile the d/128 chunk matmuls
accumulate into PSUM fp32 against the resident bf16 projection chunks; PSUM
is copied to SBUF (alternating DVE/ACT) and written out partition-major with
a graduated store schedule (small batches first so stores start early, small
last so the tail is short), alternating the two HWDGE rings. The host
scatters the returned rows to their token positions while assembling the
full output (the unshard step).
"""

import sys

import numpy as np

try:
    import concourse  # noqa: F401
except ImportError:
    sys.path.insert(0, "/opt/trn_rl_repo")

import ml_dtypes
from concourse import bacc, mybir, tile
from concourse.bass_utils import run_bass_kernel_spmd

BUCKETS = (0, 300, 3000, 30000, 267734)
SIZES = [BUCKETS[i + 1] - BUCKETS[i] for i in range(4)]
D = 1024
DS = [1024, 512, 256, 128]  # embedding dim per bucket
KS = [8, 4, 2, 1]  # 128-chunks per bucket
WOFF = [0, 8, 12, 14]  # chunk offset of each bucket in wcat
NCHUNK = 15
SUB = 32768  # rows addressable by one int16 gather call
NCORES = 8
SEQ = 4096
NTOK = NCORES * SEQ
P = 128
GB = 8  # tiles per store batch

MODE = "seq_bf16"

_BF16 = ml_dtypes.bfloat16

_cache: dict = {}


def _r16(v):
    return -(-int(v) // 16) * 16


def _r128(v):
    return -(-int(v) // 128) * 128


class Plan:
    pass


def _plan(x):
    """Global bucketing + even dealing of each bucket across the cores.

    Bucket 3 (237k rows) is dealt as contiguous runs of the index-sorted
    token list, so each core's gather indices span < 32k table rows and fit
    int16 against a per-core window of the table (passed as that core's e3
    input). Produces identical compile-time structure for all cores."""
    xf = x.reshape(-1).astype(np.int64)
    assert xf.shape[0] == NTOK
    bkt = np.searchsorted(np.asarray(BUCKETS), xf, side="right") - 1
    bkt = np.clip(bkt, 0, 3)
    loc = xf - np.asarray(BUCKETS)[bkt]

    # per-(bucket, core) token positions: sort by table index, deal
    # contiguous runs (counts differ by <=1, spans stay narrow for bucket 3)
    per_core_pos = {}
    wbase = np.zeros((4, NCORES), np.int64)  # per-core table window base
    alloc = [0] * 4
    wrows = [0] * 4  # table window rows (compile-time shape)
    for b in range(4):
        pos = np.nonzero(bkt == b)[0]
        pos = pos[np.argsort(loc[pos], kind="stable")]
        n = pos.size
        cnt = np.full(NCORES, n // NCORES)
        cnt[: n % NCORES] += 1
        cuts = np.concatenate([[0], np.cumsum(cnt)])

        def spans(cuts_):
            sp, mx = 0, 0
            for c in range(NCORES):
                pc = pos[cuts_[c] : cuts_[c + 1]]
                if pc.size:
                    sp = max(sp, int(loc[pc[-1]] - loc[pc[0]]) + 1)
                    mx = max(mx, pc.size)
            return sp, mx

        span, mxc = spans(cuts)
        if b == 3 and span > SUB:
            # skewed distribution: balanced cuts straddle >32k-row ranges;
            # fall back to fixed 32k-row boundary cuts (unbalanced counts
            # but indices stay int16 against each core's window)
            edges = np.searchsorted(loc[pos], np.arange(1, NCORES) * SUB)
            cuts = np.concatenate([[0], edges, [n]])
            span, mxc = spans(cuts)
        for c in range(NCORES):
            pc = pos[cuts[c] : cuts[c + 1]]
            per_core_pos[(b, c)] = pc
            if pc.size:
                wbase[b, c] = loc[pc[0]]
        alloc[b] = int(_r16(mxc))
        wrows[b] = min(span if b == 3 else SIZES[b], SIZES[b])
        wrows[b] = max(wrows[b], 1)
        assert wrows[b] <= SUB, (b, wrows[b])
        if b < 3:
            wbase[b] = 0

    # gather layout: one 128-aligned block per bucket (gidx/gt addressing)
    segs = []  # (bucket, o_gather, n_alloc, num_idxs)
    blocks = []
    o = 0
    for b in range(4):
        ni = _r128(alloc[b])
        segs.append((b, o, alloc[b], ni))
        blocks.append((o, ni))
        o += ni
    ntot = o

    # slot (output) layout: 64-aligned packed blocks, bucket 3 first, so at
    # most one tile boundary mixes two buckets and pad tiles are eliminated
    slot_blocks = []  # (bucket, slot_base, slot_size) in processing order
    so = 0
    for b in (3, 2, 1, 0):
        sz = -(-alloc[b] // 64) * 64
        slot_blocks.append((b, so, sz))
        so += sz
    ntot_s = _r128(so)

    p = Plan()
    p.segs, p.blocks, p.ntot = segs, blocks, ntot
    p.t_total = ntot // P
    p.slot_blocks, p.ntot_s = slot_blocks, ntot_s
    p.t_total_s = ntot_s // P
    p.alloc = alloc
    p.wrows = wrows
    p.wbase = wbase

    gidx = np.zeros((NCORES, P, ntot // 16), np.int16)
    rowpos = np.full((NCORES, ntot), -1, np.int64)  # gslot -> global token pos
    rowpos_s = np.full((NCORES, ntot_s), -1, np.int64)  # packed slot -> pos
    sbase = {b: sb for b, sb, _ in slot_blocks}
    for b, o, na, ni in segs:
        for c in range(NCORES):
            pos = per_core_pos[(b, c)]
            n = pos.size
            li = np.zeros(na, np.int64)
            li[:n] = loc[pos] - wbase[b, c]
            rowpos[c, o : o + n] = pos
            rowpos_s[c, sbase[b] : sbase[b] + n] = pos
            ii = np.arange(na)
            cols = o // 16 + ii // 16
            rows = ii % 16
            for g in range(8):  # replicate across the 8 groups of 16 partitions
                gidx[c, g * 16 + rows, cols] = li.astype(np.int16)
    p.gidx, p.rowpos, p.rowpos_s = gidx, rowpos, rowpos_s
    return p


def _build(plan, mode=MODE, repeat=1, loop_n=None, b3_first=True, split_w=True, gbatch=16, zbufs=5, psbufs=4, store_split=True, tail_dve=True, ab_no_store=False, ab_no_gather=False, ab_no_mm=False, ab_no_copy=False, g_transpose=True, g_cap=768, g_sp=True, g_q=4, g_head=128, ps_half=False, tail_split=0, ab_same_w=False, copy3=False, gsched=(1, 1, 2, 4, 8, 8, 8, 8, 4, 2), unroll=1, gbufs=2, pack64=True):
    """Build + compile the SPMD Bass program.

    repeat>1 re-emits the whole body; loop_n wraps the body in a HW For_i
    loop (both used only for differential timing)."""
    ntot, t_total = plan.ntot, plan.t_total
    bf16 = mybir.dt.bfloat16
    f32 = mybir.dt.float32
    odt = bf16 if mode.endswith("bf16") else f32

    assert 8 % g_q == 0, "g_q must divide the 8 DMASW sem lanes"
    nc = bacc.Bacc(None, target_bir_lowering=False, num_swdge_queues=g_q)
    # Pool-engine DMA instructions take DMASW sem lanes round-robin (8 lanes);
    # the sim (and sane HW sync) requires each lane to stay on ONE SWDGE
    # queue, so derive each gather's queue from the lane it will be assigned.
    pool_dma_n = [0]

    def _pool_q():
        q = (pool_dma_n[0] % 8) % g_q
        pool_dma_n[0] += 1
        return q
    e_d = [
        nc.declare_dram_parameter(f"e{b}", [plan.wrows[b], DS[b]], bf16, isOutput=False)
        for b in range(4)
    ]
    wcat_d = nc.declare_dram_parameter("wcat", [P, NCHUNK * D], bf16, isOutput=False)
    gidx_d = nc.declare_dram_parameter("gidx", [P, ntot // 16], mybir.dt.int16, isOutput=False)
    # partition-major: slot s lives at out[s % 128, s // 128, :] so each
    # partition's store stream is contiguous (few, large descriptors)
    t_out = plan.t_total_s if pack64 else t_total
    out_d = nc.declare_dram_parameter("out", [P, t_out, D], odt, isOutput=True)

    bbase = [blk[0] for blk in plan.blocks]
    bslots = [blk[1] for blk in plan.blocks]

    with tile.TileContext(nc) as tc:
        with (
            tc.tile_pool(name="const", bufs=1) as cp,
            tc.tile_pool(name="gbuf", bufs=gbufs) as gp,
            tc.tile_pool(name="zbuf", bufs=zbufs) as zp,
            tc.tile_pool(name="ps", bufs=psbufs, space="PSUM") as pp,
        ):
            gidx = cp.tile([P, ntot // 16], mybir.dt.int16)
            nc.gpsimd.dma_start(out=gidx[:], in_=gidx_d[:])
            _pool_q()  # gidx load consumes a DMASW lane
            wcat = cp.tile([P, NCHUNK * D], bf16)
            if split_w:
                # W3 first: bucket-3 tiles are processed first and its W slice
                # is small, so the first matmuls aren't gated on the full load
                upfront = (3,) if split_w == 3 else (3, 2, 1, 0)
                for b in upfront:
                    sl = slice(WOFF[b] * D, (WOFF[b] + KS[b]) * D)
                    eng = nc.sync if (split_w is True or split_w in (1, 3) or b >= 2) else nc.scalar
                    eng.dma_start(out=wcat[:, sl], in_=wcat_d[:, sl])
            else:
                nc.sync.dma_start(out=wcat[:], in_=wcat_d[:])



            def bucket_of_tile(t):
                slot = t * P
                for b in range(4):
                    if bbase[b] <= slot < bbase[b] + bslots[b]:
                        return b
                raise AssertionError(t)

            def segments_of_tile(t):
                # (bucket, in-bucket offset, psum partition offset, ncols)
                if not pack64:
                    b = bucket_of_tile(t)
                    return [(b, t * P - bbase[b], 0, P)]
                out = []
                for b, sb, sz in plan.slot_blocks:
                    lo = max(t * P, sb)
                    hi = min((t + 1) * P, sb + sz)
                    if lo < hi:
                        out.append((b, lo - sb, lo - t * P, hi - lo))
                return out

            def body(_iv=None):
                gt = [
                    gp.tile([P, KS[b], bslots[b]], bf16, name=f"G{b}", tag=f"G{b}")
                    if bslots[b]
                    else None
                    for b in range(4)
                ]
                if ab_no_gather and not ab_no_mm:
                    for b in range(4):  # dummy-allocate gt for the ablation
                        nc.vector.memset(gt[b][:, :, :1], 0)
                deferred_w = []
                if split_w == 3:
                    for b in (2, 1, 0):
                        sl = slice(WOFF[b] * D, (WOFF[b] + KS[b]) * D)
                        deferred_w.append(sl)
                GCAP = g_cap  # >=1024 idxs in one SWDGE gather wedges the device
                gcall = 0
                gsegs = (
                    sorted(plan.segs, key=lambda e: e[0] != 3)
                    if b3_first
                    else plan.segs
                )

                def chunks_for(b, ni, first):
                    if KS[b] != 1:
                        return [(0, ni)]
                    out, k = [], 0
                    if first and g_head:
                        # one small head call per SWDGE queue so the first
                        # tiles land fast on all queues concurrently
                        for _ in range(g_q):
                            if k + g_head > ni:
                                break
                            out.append((k, g_head))
                            k += g_head
                    while k < ni:
                        nk = min(GCAP, ni - k)
                        out.append((k, nk))
                        k += nk
                    return out

                for si, (b, o, na, ni) in enumerate(gsegs):
                    if ab_no_gather:
                        break
                    for k, nk in chunks_for(b, ni, si == 0):
                        ok = o + k
                        o_local = ok - bbase[b]
                        if g_transpose:
                            out_ap = gt[b][:, :, o_local : o_local + nk]
                        else:
                            gnt = gp.tile(
                                [P, nk // P, DS[b]],
                                bf16,
                                name=f"GN{gcall}",
                                tag=f"GN{gcall}",
                            )
                            out_ap = gnt[:]
                        nc.gpsimd.dma_gather(
                            out_ap=out_ap,
                            in_ap=e_d[b][:],
                            idxs_ap=gidx[:, ok // 16 : ok // 16 + nk // 16],
                            num_idxs=nk,
                            num_idxs_reg=nk,
                            elem_size=DS[b],
                            transpose=g_transpose,
                            single_packet=g_sp,
                            queue_num=_pool_q(),
                        )
                        gcall += 1

                # bucket-3 tiles first (largest block, cheapest W), then 0..2
                if pack64:
                    order = list(range(plan.t_total_s))  # b3-first by layout
                else:
                    t3 = bbase[3] // P
                    if b3_first:
                        order = list(range(t3, t_total)) + list(range(0, t3))
                    else:
                        order = list(range(t_total))
                oi = 0
                batch_i = 0
                while oi < len(order):
                    t = order[oi]
                    rem = len(order) - oi
                    if gsched is not None:
                        eff_gb = gsched[batch_i] if batch_i < len(gsched) else gsched[-1]
                    else:
                        eff_gb = 2 if (tail_split and rem <= tail_split) else gbatch
                    batch_i += 1
                    gb = 1
                    for g in range(1, min(eff_gb, rem)):
                        if order[oi + g] == t + g:
                            gb += 1
                        else:
                            break
                    zt = zp.tile([P, gb, D], odt, tag="z")
                    if ab_no_mm or ab_no_copy:
                        nc.vector.memset(zt[:, :, :1], 0)
                    for g in range(gb if not ab_no_mm else 0):
                        tt = t + g
                        if not pack64:
                            b = bucket_of_tile(tt)
                            ts0 = tt * P - bbase[b]
                            kb = KS[b]
                        else:
                            assert not ps_half
                        if ps_half:
                            ps0 = pp.tile([P, 512], f32, tag="ps0")
                            ps1 = pp.tile([P, 512], f32, tag="ps1")
                            pss = (ps0, ps1)
                            for c in range(kb):
                                lhsT = gt[b][:, c, ts0 : ts0 + P]
                                for h in range(2):
                                    nc.tensor.matmul(
                                        out=pss[h][:, :],
                                        lhsT=lhsT,
                                        rhs=wcat[:, (WOFF[b] + c) * D + h * 512 :][:, :512],
                                        start=(c == 0),
                                        stop=(c == kb - 1),
                                    )
                            if not ab_no_copy:
                                nc.vector.tensor_copy(out=zt[:, g, :512], in_=ps0[:])
                                nc.scalar.copy(out=zt[:, g, 512:], in_=ps1[:])
                            oi_done = True
                        else:
                            ps = pp.tile([P, D], f32, tag="ps")
                            for b2_, boff, m0, ncols in segments_of_tile(tt):
                                kb2 = KS[b2_]
                                for c in range(kb2):
                                    lhsT = (
                                        gt[3][:, 0, 0:P]
                                        if ab_same_w
                                        else gt[b2_][:, c, boff : boff + ncols]
                                    )
                                    for h in range(2):
                                        nc.tensor.matmul(
                                            out=ps[m0 : m0 + ncols, h * 512 : (h + 1) * 512],
                                            lhsT=lhsT,
                                            rhs=wcat[:, (WOFF[b2_] + c) * D + h * 512 :][:, :512],
                                            start=(c == 0),
                                            stop=(c == kb2 - 1),
                                        )
                            last_batch = oi + gb >= len(order)
                            if ab_no_copy:
                                pass
                            elif copy3:
                                cengs = (nc.vector, nc.scalar, nc.gpsimd)
                                for h in range(2):
                                    eng = cengs[(tt * 2 + h) % 3]
                                    if eng is nc.scalar:
                                        eng.copy(
                                            out=zt[:, g, h * 512 : (h + 1) * 512],
                                            in_=ps[:, h * 512 : (h + 1) * 512],
                                        )
                                    else:
                                        eng.tensor_copy(
                                            out=zt[:, g, h * 512 : (h + 1) * 512],
                                            in_=ps[:, h * 512 : (h + 1) * 512],
                                        )
                            elif tail_dve == 2 and not last_batch:
                                nc.vector.tensor_copy(out=zt[:, g, :512], in_=ps[:, :512])
                                nc.scalar.copy(out=zt[:, g, 512:], in_=ps[:, 512:])
                            elif tt % 2 == 0 or (tail_dve and last_batch):
                                nc.vector.tensor_copy(out=zt[:, g, :], in_=ps[:])
                            else:
                                nc.scalar.copy(out=zt[:, g, :], in_=ps[:])
                    last_batch_s = oi + gb >= len(order)
                    if store_split == 2:
                        seng = nc.scalar if last_batch_s else nc.sync
                    else:
                        seng = nc.scalar if (store_split and (t // gbatch) % 2) else nc.sync
                    if not ab_no_store:
                        seng.dma_start(out=out_d[:, t : t + gb, :], in_=zt[:])
                    for sl in deferred_w:
                        nc.sync.dma_start(out=wcat[:, sl], in_=wcat_d[:, sl])
                    deferred_w = []
                    oi += gb

            if loop_n is None:
                for _ in range(repeat):
                    body()
            else:
                with tc.For_i(0, loop_n, 1) as _i:
                    for _ in range(unroll):
                        body()
    nc.compile()
    return nc


def _prep_inputs(embs, ws, plan, mode=MODE):
    wcat = np.zeros((P, NCHUNK * D), _BF16)
    for b in range(4):
        for c in range(KS[b]):
            wcat[:, (WOFF[b] + c) * D : (WOFF[b] + c + 1) * D] = ws[b][
                c * P : (c + 1) * P, :
            ].astype(_BF16)
    ebf = [e.astype(_BF16) for e in embs]
    in_maps = []
    for c in range(NCORES):
        m = {}
        for b in range(4):
            base = int(plan.wbase[b, c])
            w = plan.wrows[b]
            win = ebf[b][base : base + w]
            if win.shape[0] < w:  # window runs past the table end: zero-pad
                win = np.concatenate(
                    [win, np.zeros((w - win.shape[0], DS[b]), _BF16)]
                )
            m[f"e{b}"] = np.ascontiguousarray(win)
        m["wcat"] = wcat
        m["gidx"] = np.ascontiguousarray(plan.gidx[c])
        in_maps.append(m)
    return in_maps


def _assemble(plan, mode, results, repeat=1):
    out = np.empty((NTOK, D), np.float32)
    for c in range(NCORES):
        r = results[c]["out"]  # [128, T, D] partition-major
        packed = r.shape[1] == plan.t_total_s
        rp = plan.rowpos_s if packed else plan.rowpos
        r = np.ascontiguousarray(r.transpose(1, 0, 2)).reshape(-1, D)
        valid = rp[c] >= 0
        out[rp[c][valid]] = r[valid].astype(np.float32)
    return out.reshape(NCORES, SEQ, D)


def run(inputs, mode=MODE, trace=False):
    x = np.asarray(inputs["x"])
    embs = [np.asarray(inputs[f"emb{b}"]) for b in range(4)]
    ws = [np.asarray(inputs[f"W{b}"]) for b in range(4)]
    assert x.shape == (NCORES, SEQ), x.shape

    plan = _plan(x)
    key = (tuple(plan.alloc), tuple(plan.wrows), mode)
    if key not in _cache:
        _cache[key] = _build(plan, mode)
    nc = _cache[key]

    in_maps = _prep_inputs(embs, ws, plan, mode)
    res = run_bass_kernel_spmd(
        nc, in_maps, core_ids=list(range(NCORES)), trace=trace
    )
    out = _assemble(plan, mode, res.results)
    return out, res


def kernel(**inputs):
    out, _ = run(inputs, mode=MODE, trace=False)
    return out

